# Initial kernel scaffold
#
"""Dilated block attention + output projection on 8 trn2 cores.

Sharding: core c handles batch b = c//2 and heads h = 4*(c%2) .. +3.
Each core computes the full dilated-attention combine for its 4 (b,h)
pairs and a partial output projection (contraction over its 4 heads'
256 hidden dims).  The host sums the two half-hidden partials per batch
and adds the bias.

Math note: the reference's stabilized-softmax + detached-expsum
reweighting collapses to the unstabilized form
    out[p] = (sum_d exp(S_d) @ V_d  scattered to p) / (sum_d rowsum exp(S_d))
which is what the kernel computes (scores ~ N(0,1), no overflow risk).

Device layout per (b,h):
  - S^T tiles ([keys, q]) from matmul(lhsT=K^T[64,128], rhs=Q^T[64,512]);
    row groups alternate partitions 0-63 / 64-127 per k-tile so pairs of
    K=64 matmuls run concurrently in the PE array.
  - exp on ScalarE (PSUM -> SBUF), scale=0.125 folds the 1/sqrt(hd).
  - PV: matmul(lhsT=V'[128,65], rhs=E[128,512]) accumulated over k-tiles;
    V' carries a ones column so row 64 of the psum is the exp row-sum.
  - DVE scatter-adds branch windows into per-head accumulators [65, 4096].
  - 1/w via custom-DVE fast reciprocal, broadcast across partitions on
    GpSimd, multiplied into the accumulator.
  - o_proj: per M-tile, 4 accumulating K=64 matmuls against Wo^T slices.
"""

import numpy as np

B, H, L, HD = 4, 8, 4096, 64
HIDDEN = H * HD
DILS = (1, 2, 4, 8)
BLOCK = 1024
PB = 4  # (b,h) pairs per core
NCORES = 8
LDS = [L // d for d in DILS]  # 4096, 2048, 1024, 512
OFFS = [0, 4096, 6144, 7168]
LSUM = sum(LDS)  # 7680
QCH = 512  # q-chunk (positions of the strided domain) per psum window

_PROGRAM = None


def build_program():
    """Build the (SPMD, identical on all cores) Bass program."""
    from contextlib import ExitStack

    import concourse.bass as bass
    import concourse.tile as tile
    from concourse import mybir

    F32 = mybir.dt.float32
    nc = bass.Bass()

    qt_d = nc.dram_tensor("qt", [PB, 64, LSUM], F32, kind="ExternalInput")
    kt_d = nc.dram_tensor("kt", [PB, 64, LSUM], F32, kind="ExternalInput")
    v_d = nc.dram_tensor("v", [PB, LSUM, 64], F32, kind="ExternalInput")
    wot_d = nc.dram_tensor("wot", [PB, 64, HIDDEN], F32, kind="ExternalInput")
    out_d = nc.dram_tensor("out", [L, HIDDEN], F32, kind="ExternalOutput")

    with tile.TileContext(nc) as tc, ExitStack() as ctx:
        consts = ctx.enter_context(tc.tile_pool(name="consts", bufs=1))
        qk_pool = ctx.enter_context(tc.tile_pool(name="qk", bufs=1))
        v_pool = ctx.enter_context(tc.tile_pool(name="vp", bufs=1))
        e_pool = ctx.enter_context(tc.tile_pool(name="ep", bufs=3))
        acc_pool = ctx.enter_context(tc.tile_pool(name="accp", bufs=1))
        io_pool = ctx.enter_context(tc.tile_pool(name="iop", bufs=2))
        st_psum = ctx.enter_context(tc.tile_pool(name="stp", bufs=2, space="PSUM"))
        pv_psum = ctx.enter_context(tc.tile_pool(name="pvp", bufs=2, space="PSUM"))

        zero_bias = consts.tile([128, 1], F32, tag="zb")
        nc.vector.memset(zero_bias, 0.0)

        wot_sb = consts.tile([64, PB, HIDDEN], F32, tag="wot")
        for j in range(PB):
            nc.sync.dma_start(out=wot_sb[:, j, :], in_=wot_d[j])

        acc_tiles = [
            acc_pool.tile([65, L], F32, tag=f"acc{j}", bufs=1) for j in range(PB)
        ]

        for j in range(PB):
            acc = acc_tiles[j]
            for di, d in enumerate(DILS):
                Ld, off = LDS[di], OFFS[di]
                bs = min(BLOCK, Ld)
                nblk = Ld // bs
                nkt_blk = bs // 128
                nkt_br = Ld // 128

                # Q^T duplicated on both partition halves (rhs for both
                # row groups).  d=1 single-buffered to fit SBUF.
                qbufs = 1 if d == 1 else 2
                qtile = qk_pool.tile([128, Ld], F32, tag=f"qt{di}", bufs=qbufs)
                src_q = qt_d[j, :, off : off + Ld]
                nc.sync.dma_start(out=qtile[0:64, :], in_=src_q)
                nc.sync.dma_start(out=qtile[64:128, :], in_=src_q)

                # K^T split: even k-tiles on partitions 0-63, odd on 64-127.
                ktile = qk_pool.tile(
                    [128, nkt_br // 2 if nkt_br > 1 else 1, 128],
                    F32,
                    tag=f"kt{di}",
                    bufs=2,
                )
                src_k = kt_d[j, :, off : off + Ld]
                if nkt_br > 1:
                    k3 = src_k.rearrange("r (t c) -> r t c", c=128)
                    nc.sync.dma_start(out=ktile[0:64, :, :], in_=k3[:, 0::2, :])
                    nc.sync.dma_start(out=ktile[64:128, :, :], in_=k3[:, 1::2, :])
                else:
                    nc.sync.dma_start(out=ktile[0:64, 0, :], in_=src_k)

                # V with a ones column appended per k-tile slab.
                vtile = v_pool.tile(
                    [128, nkt_br, 65], F32, tag=f"v{di}", bufs=(1 if d == 1 else 2)
                )
                v3 = v_d[j, off : off + Ld, :].rearrange("(t p) c -> p t c", p=128)
                nsplit = 4 if d == 1 else 1
                step = nkt_br // nsplit
                for s in range(nsplit):
                    nc.sync.dma_start(
                        out=vtile[:, s * step : (s + 1) * step, 0:64],
                        in_=v3[:, s * step : (s + 1) * step, :],
                    )
                nc.vector.memset(vtile[:, :, 64:65], 1.0)

                for blk in range(nblk):
                    for qc in range(bs // QCH):
                        q0 = blk * bs + qc * QCH
                        pv = pv_psum.tile([128, QCH], F32, tag="pv")
                        kts = list(range(nkt_blk))
                        if nkt_blk == 8:
                            groups = [kts[0:3], kts[3:6], kts[6:8]]
                        else:
                            groups = [kts[0:2], kts[2:4]]
                        done = 0
                        for g in groups:
                            gs = len(g)
                            st = st_psum.tile([128, 3, QCH], F32, tag="st")
                            et = e_pool.tile([128, 3, QCH], F32, tag="et")
                            for i, kt in enumerate(g):
                                half = kt % 2
                                nc.tensor.matmul(
                                    st[:, i, :],
                                    ktile[
                                        half * 64 : (half + 1) * 64,
                                        (blk * nkt_blk + kt) // 2,
                                        :,
                                    ],
                                    qtile[
                                        half * 64 : (half + 1) * 64, q0 : q0 + QCH
                                    ],
                                    start=True,
                                    stop=True,
                                )
                            nc.scalar.activation(
                                et[:, 0:gs, :],
                                st[:, 0:gs, :],
                                mybir.ActivationFunctionType.Exp,
                                bias=zero_bias,
                                scale=0.125,
                            )
                            for i, kt in enumerate(g):
                                nc.tensor.matmul(
                                    pv[0:65, :],
                                    vtile[:, blk * nkt_blk + kt, :],
                                    et[:, i, :],
                                    start=(done == 0),
                                    stop=(done == nkt_blk - 1),
                                )
                                done += 1
                        # scatter/accumulate the window into acc
                        p0 = q0 * d
                        if d == 1:
                            nc.vector.tensor_copy(
                                out=acc[:, p0 : p0 + QCH], in_=pv[0:65, :]
                            )
                        else:
                            dst = acc[:, p0 : p0 + QCH * d : d]
                            nc.vector.tensor_add(
                                out=dst, in0=dst, in1=pv[0:65, :]
                            )

            # normalize: acc[0:64, :] /= acc[64, :]
            for w in range(L // QCH):
                ws = slice(w * QCH, (w + 1) * QCH)
                wr = io_pool.tile([1, QCH], F32, tag="wr")
                nc.sync.dma_start(out=wr, in_=acc[64:65, ws])
                nc.vector.reciprocal_approx_fast(out=wr, in_=wr)
                rb = io_pool.tile([64, QCH], F32, tag="rb")
                nc.gpsimd.partition_broadcast(rb, wr, channels=64)
                nc.vector.tensor_mul(
                    out=acc[0:64, ws], in0=acc[0:64, ws], in1=rb
                )

        # partial o_proj: out[p, :] = sum_j acc_j[:, p]^T @ wot_j
        for mt in range(L // 128):
            po = pv_psum.tile([128, HIDDEN], F32, tag="po")
            for j in range(PB):
                nc.tensor.matmul(
                    po,
                    acc_tiles[j][0:64, mt * 128 : (mt + 1) * 128],
                    wot_sb[:, j, :],
                    start=(j == 0),
                    stop=(j == PB - 1),
                )
            ot = io_pool.tile([128, HIDDEN], F32, tag="ot")
            nc.vector.tensor_copy(out=ot, in_=po)
            nc.sync.dma_start(out=out_d[mt * 128 : (mt + 1) * 128, :], in_=ot)

    return nc


def get_program():
    global _PROGRAM
    if _PROGRAM is None:
        _PROGRAM = build_program()
    return _PROGRAM


def make_in_maps(query_states, key_states, value_states, Wo):
    q = np.ascontiguousarray(np.asarray(query_states, dtype=np.float32))
    k = np.ascontiguousarray(np.asarray(key_states, dtype=np.float32))
    v = np.ascontiguousarray(np.asarray(value_states, dtype=np.float32))
    Wo = np.asarray(Wo, dtype=np.float32)

    in_maps = []
    for c in range(NCORES):
        b, hs = c // 2, (c % 2) * PB
        qt = np.empty((PB, 64, LSUM), np.float32)
        kt = np.empty((PB, 64, LSUM), np.float32)
        vv = np.empty((PB, LSUM, 64), np.float32)
        wot = np.empty((PB, 64, HIDDEN), np.float32)
        for j in range(PB):
            h = hs + j
            for di, d in enumerate(DILS):
                off, Ld = OFFS[di], LDS[di]
                qt[j, :, off : off + Ld] = q[b, h, ::d, :].T
                kt[j, :, off : off + Ld] = k[b, h, ::d, :].T
                vv[j, off : off + Ld, :] = v[b, h, ::d, :]
            wot[j] = Wo[:, h * 64 : (h + 1) * 64].T
        in_maps.append({"qt": qt, "kt": kt, "v": vv, "wot": wot})
    return in_maps


def combine_outputs(results, bo):
    bo = np.asarray(bo, dtype=np.float32)
    out = np.empty((B, L, HIDDEN), np.float32)
    for b in range(B):
        out[b] = results[2 * b]["out"] + results[2 * b + 1]["out"] + bo
    return out


def kernel(
    query_states, key_states, value_states, Wo, bo, _trace=False, _results=[None]
):
    from concourse.bass_utils import run_bass_kernel_spmd

    nc = get_program()
    in_maps = make_in_maps(query_states, key_states, value_states, Wo)
    res = run_bass_kernel_spmd(nc, in_maps, list(range(NCORES)), trace=_trace)
    _results[0] = res
    return combine_outputs(res.results, bo)


# revision 21
# speedup vs baseline: 2.9355x; 2.9355x over previous
"""Dilated block attention + output projection on 8 trn2 cores.

Sharding: core c handles batch b = c//2 and heads h = 4*(c%2) .. +3.
Each core computes the full dilated-attention combine for its 4 (b,h)
pairs and a partial output projection (contraction over its 4 heads'
256 hidden dims).  The host sums the two half-hidden partials per batch
and adds the bias.

Math note: the reference's stabilized-softmax + detached-expsum
reweighting collapses to the unstabilized form
    out[p] = (sum_d exp(S_d) @ V_d  scattered to p) / (sum_d rowsum exp(S_d))
which is what the kernel computes (scores ~ N(0,1), no overflow risk).

Device layout per (b,h), per dilation branch: the host packs ONE blob
[128, W] per branch holding, in SBUF layout:
  - Q^T [64, Ld] duplicated onto both partition halves (matmul rhs for
    both PE row groups),
  - K^T k-tiles parity-split: even k-tiles on partitions 0-63, odd on
    64-127 (so consecutive K=64 QK matmuls land on different PE row
    groups and run concurrently),
  - V k-tile slabs [128, 65] with a ones column (PV matmul with M=65
    gives the exp row-sum on psum row 64 for free).
One DMA per branch.  S^T = matmul(lhsT=K^T[64,128], rhs=Q^T[64,512]) to
PSUM; exp on ScalarE (PSUM->SBUF, scale=0.125 folds 1/sqrt(hd)); PV
accumulates over k-tiles into a [65, 512] psum window; DVE scatter-adds
windows into per-head accumulators [65, 4096]; 1/w via custom-DVE fast
reciprocal + K=1 ones-matmul partition broadcast; o_proj as 4
accumulating K=64 matmuls per M-tile against Wo^T slices.

Matmul operands are bf16 (fp32 matmuls run as two PE passes on trn2);
psum accumulation and the softmax combine stay fp32.  The PE stream is
software-pipelined (QK/exp of group i+1 issued before PV of group i) so
the in-order PE queue never head-of-line blocks on the ScalarE exp.
"""

import ml_dtypes
import numpy as np

BF16_NP = ml_dtypes.bfloat16

B, H, L, HD = 4, 8, 4096, 64
HIDDEN = H * HD
DILS = (1, 2, 4, 8)
BLOCK = 1024
PB = 4  # (b,h) pairs per core
NCORES = 8
LDS = [L // d for d in DILS]  # 4096, 2048, 1024, 512
NKTS = [ld // 128 for ld in LDS]  # 32, 16, 8, 4
# blob widths per branch: Q dup (Ld) + K parity-split (Ld/2) + V slabs (nkt*65)
WS = [ld + ld // 2 + nkt * 65 for ld, nkt in zip(LDS, NKTS)]
BOFFS = [sum(WS[:i]) for i in range(len(WS))]
WSUM = sum(WS)
QCH = 512  # q-chunk (strided-domain positions) per psum window

_PROGRAM = None


def build_program():
    """Build the (SPMD, identical on all cores) Bass program."""
    from contextlib import ExitStack

    import concourse.tile as tile
    from concourse import bacc, mybir

    F32 = mybir.dt.float32
    BF16 = mybir.dt.bfloat16
    nc = bacc.Bacc("TRN2", target_bir_lowering=False, debug=False)

    blob_d = nc.dram_tensor("blob", [PB, 128, WSUM], BF16, kind="ExternalInput")
    wot_d = nc.dram_tensor("wot", [PB, 64, HIDDEN], BF16, kind="ExternalInput")
    out_d = nc.dram_tensor("out", [L, HIDDEN], F32, kind="ExternalOutput")

    with tile.TileContext(nc) as tc, ExitStack() as ctx:
        consts = ctx.enter_context(tc.tile_pool(name="consts", bufs=1))
        br_pool = ctx.enter_context(tc.tile_pool(name="br", bufs=1))
        e_pool = ctx.enter_context(tc.tile_pool(name="ep", bufs=4))
        acc_pool = ctx.enter_context(tc.tile_pool(name="accp", bufs=1))
        io_pool = ctx.enter_context(tc.tile_pool(name="iop", bufs=2))
        st_psum = ctx.enter_context(tc.tile_pool(name="stp", bufs=2, space="PSUM"))
        pv_psum = ctx.enter_context(tc.tile_pool(name="pvp", bufs=2, space="PSUM"))

        zero_bias = consts.tile([128, 1], F32, tag="zb")
        nc.vector.memset(zero_bias, 0.0)
        ones_row = consts.tile([1, 64], F32, tag="ones_row")
        nc.vector.memset(ones_row, 1.0)

        wot_sb = consts.tile([64, PB, HIDDEN], BF16, tag="wot")
        nc.sync.dma_start(out=wot_sb, in_=wot_d.rearrange("j r c -> r j c"))

        acc_tiles = [
            acc_pool.tile([65, L], F32, tag=f"acc{j}", bufs=1, name=f"acc{j}")
            for j in range(PB)
        ]
        oacc_tiles = [
            acc_pool.tile([64, L], BF16, tag=f"oacc{j}", bufs=1, name=f"oacc{j}")
            for j in range(PB)
        ]

        for j in range(PB):
            acc = acc_tiles[j]

            # Build the flat job list: one job per (branch, window, k-group).
            jobs = []
            bt_tiles = {}
            for di, d in enumerate(DILS):
                Ld = LDS[di]
                bs = min(BLOCK, Ld)
                nblk = Ld // bs
                nkt_blk = bs // 128
                for blk in range(nblk):
                    for qc in range(bs // QCH):
                        q0 = blk * bs + qc * QCH
                        kts = list(range(nkt_blk))
                        groups = (
                            [kts[0:3], kts[3:6], kts[6:8]]
                            if nkt_blk == 8
                            else [kts[0:2], kts[2:4]]
                        )
                        for gi, g in enumerate(groups):
                            jobs.append(
                                dict(
                                    di=di,
                                    d=d,
                                    blk=blk,
                                    nkt_blk=nkt_blk,
                                    q0=q0,
                                    g=g,
                                    first=(gi == 0),
                                    last=(gi == len(groups) - 1),
                                    done0=sum(len(x) for x in groups[:gi]),
                                )
                            )

            def get_bt(di):
                if di not in bt_tiles:
                    bufs = 1 if di <= 1 else 2
                    bt = br_pool.tile(
                        [128, WS[di]], BF16, tag=f"b{di}", bufs=bufs, name=f"bt{di}"
                    )
                    nc.sync.dma_start(
                        out=bt, in_=blob_d[j, :, BOFFS[di] : BOFFS[di] + WS[di]]
                    )
                    bt_tiles[di] = bt
                return bt_tiles[di]

            # prefetch the first branches
            get_bt(0)
            get_bt(1)

            def emit_qk_exp(job):
                """QK matmuls for the group -> exp to a bf16 E tile."""
                di, q0, g = job["di"], job["q0"], job["g"]
                Ld = LDS[di]
                kbase = Ld
                bt = get_bt(di)
                gs = len(g)
                st = st_psum.tile([128, 3, QCH], F32, tag="st", name="st")
                for i, kt in enumerate(g):
                    tg = job["blk"] * job["nkt_blk"] + kt
                    half = tg % 2
                    k0 = kbase + (tg // 2) * 128
                    nc.tensor.matmul(
                        st[:, i, :],
                        bt[half * 64 : (half + 1) * 64, k0 : k0 + 128],
                        bt[half * 64 : (half + 1) * 64, q0 : q0 + QCH],
                        start=True,
                        stop=True,
                    )
                et = e_pool.tile([128, 3, QCH], BF16, tag="et", name="et")
                nc.scalar.activation(
                    et[:, 0:gs, :],
                    st[:, 0:gs, :],
                    mybir.ActivationFunctionType.Exp,
                    bias=zero_bias,
                    scale=0.125,
                )
                job["et"] = et

            def emit_pv(job):
                """PV accumulation for the group; combine if window done."""
                di, d = job["di"], job["d"]
                Ld = LDS[di]
                vbase = Ld + Ld // 2
                bt = get_bt(di)
                et = job["et"]
                pv = job["pv"]
                done = job["done0"]
                for i, kt in enumerate(job["g"]):
                    tg = job["blk"] * job["nkt_blk"] + kt
                    nc.tensor.matmul(
                        pv[0:65, :],
                        bt[:, vbase + tg * 65 : vbase + tg * 65 + 65],
                        et[:, i, :],
                        start=(done == 0),
                        stop=(done == job["nkt_blk"] - 1),
                        skip_group_check=True,
                    )
                    done += 1
                if job["last"]:
                    p0 = job["q0"] * d
                    if d == 1:
                        nc.vector.tensor_copy(
                            out=acc[:, p0 : p0 + QCH], in_=pv[0:65, :]
                        )
                    else:
                        dst = acc[:, p0 : p0 + QCH * d : d]
                        nc.vector.tensor_add(out=dst, in0=dst, in1=pv[0:65, :])

            # software pipeline, depth 2: QK/exp of job i, then PV of job
            # i-2, so the in-order PE queue never blocks on the ACT exp.
            from collections import deque

            pending = deque()
            cur_pv = None
            for idx, job in enumerate(jobs):
                if job["first"]:
                    cur_pv = pv_psum.tile([128, QCH], F32, tag="pv", name="pv")
                job["pv"] = cur_pv
                # prefetch next branch blob one branch ahead
                if idx > 0 and job["di"] != jobs[idx - 1]["di"] and job["di"] < 3:
                    get_bt(job["di"] + 1)
                emit_qk_exp(job)
                pending.append(job)
                if len(pending) > 2:
                    emit_pv(pending.popleft())
            while pending:
                emit_pv(pending.popleft())
            bt_tiles.clear()

            # normalize: oacc = acc[0:64, :] * (1 / acc[64, :]) in bf16.
            # Custom-DVE ops silently no-op at base partition 64, so move the
            # w row to partition 0 first (SBUF->SBUF DMA crosses partitions).
            wrow = io_pool.tile([1, L], F32, tag="wrow", bufs=1)
            nc.sync.dma_start(out=wrow, in_=acc[64:65, :])
            nc.vector.reciprocal_approx_fast(out=wrow, in_=wrow)
            for w in range(L // QCH):
                ws = slice(w * QCH, (w + 1) * QCH)
                # broadcast 1/w across 64 partitions via a K=1 ones-matmul
                bc = pv_psum.tile([64, QCH], F32, tag="pv", name="bc")
                nc.tensor.matmul(
                    bc, ones_row[0:1, :], wrow[0:1, ws], start=True, stop=True
                )
                nc.vector.tensor_mul(
                    out=oacc_tiles[j][:, ws], in0=acc[0:64, ws], in1=bc
                )

        # partial o_proj: out[p, :] = sum_j oacc_j[:, p]^T @ wot_j
        for mt in range(L // 128):
            po = pv_psum.tile([128, HIDDEN], F32, tag="pv", name="po")
            for j in range(PB):
                nc.tensor.matmul(
                    po,
                    oacc_tiles[j][:, mt * 128 : (mt + 1) * 128],
                    wot_sb[:, j, :],
                    start=(j == 0),
                    stop=(j == PB - 1),
                    skip_group_check=True,
                )
            ot = io_pool.tile([128, HIDDEN], F32, tag="ot")
            nc.vector.tensor_copy(out=ot, in_=po)
            nc.sync.dma_start(out=out_d[mt * 128 : (mt + 1) * 128, :], in_=ot)

    nc.compile()
    return nc


def get_program():
    global _PROGRAM
    if _PROGRAM is None:
        _PROGRAM = build_program()
    return _PROGRAM


def _branch_blob(qT, kT, vv, di):
    """Pack one dilation branch into the [128, W] SBUF-layout blob.

    qT, kT: [64, Ld] transposed Q/K for this branch; vv: [Ld, 65] V plus
    ones column."""
    Ld, nkt = LDS[di], NKTS[di]
    q_part = np.concatenate([qT, qT], axis=0)  # [128, Ld]
    k3 = kT.reshape(64, nkt, 128)
    k_part = np.concatenate(
        [
            k3[:, 0::2, :].reshape(64, -1),
            k3[:, 1::2, :].reshape(64, -1),
        ],
        axis=0,
    )  # [128, Ld/2]
    v_part = vv.reshape(nkt, 128, 65).transpose(1, 0, 2).reshape(128, nkt * 65)
    return np.concatenate([q_part, k_part, v_part], axis=1)


def make_in_maps(query_states, key_states, value_states, Wo):
    q = np.asarray(query_states, dtype=np.float32)
    k = np.asarray(key_states, dtype=np.float32)
    v = np.asarray(value_states, dtype=np.float32)
    Wo = np.asarray(Wo, dtype=np.float32)

    in_maps = []
    for c in range(NCORES):
        b, hs = c // 2, (c % 2) * PB
        blob = np.empty((PB, 128, WSUM), BF16_NP)
        wot = np.empty((PB, 64, HIDDEN), BF16_NP)
        for j in range(PB):
            h = hs + j
            for di, d in enumerate(DILS):
                Ld = LDS[di]
                vv = np.empty((Ld, 65), np.float32)
                vv[:, 0:64] = v[b, h, ::d, :]
                vv[:, 64] = 1.0
                blob[j, :, BOFFS[di] : BOFFS[di] + WS[di]] = _branch_blob(
                    np.ascontiguousarray(q[b, h, ::d, :].T),
                    np.ascontiguousarray(k[b, h, ::d, :].T),
                    vv,
                    di,
                )
            wot[j] = Wo[:, h * 64 : (h + 1) * 64].T
        in_maps.append({"blob": blob, "wot": wot})
    return in_maps


def combine_outputs(results, bo):
    bo = np.asarray(bo, dtype=np.float32)
    out = np.empty((B, L, HIDDEN), np.float32)
    for b in range(B):
        out[b] = results[2 * b]["out"] + results[2 * b + 1]["out"] + bo
    return out


def kernel(
    query_states,
    key_states,
    value_states,
    Wo,
    bo,
    _trace=False,
    _tmpdir=None,
    _results=[None],
):
    from concourse.bass_utils import run_bass_kernel_spmd

    nc = get_program()
    in_maps = make_in_maps(query_states, key_states, value_states, Wo)
    res = run_bass_kernel_spmd(
        nc, in_maps, list(range(NCORES)), trace=_trace, tmpdir=_tmpdir
    )
    _results[0] = res
    return combine_outputs(res.results, bo)


# revision 22
# speedup vs baseline: 3.0804x; 1.0493x over previous
"""Dilated block attention + output projection on 8 trn2 cores.

Sharding: core c handles batch b = c//2 and heads h = 4*(c%2) .. +3.
Each core computes the full dilated-attention combine for its 4 (b,h)
pairs and a partial output projection (contraction over its 4 heads'
256 hidden dims).  The host sums the two half-hidden partials per batch
and adds the bias.

Math note: the reference's stabilized-softmax + detached-expsum
reweighting collapses to the unstabilized form
    out[p] = (sum_d exp(S_d) @ V_d  scattered to p) / (sum_d rowsum exp(S_d))
which is what the kernel computes (scores ~ N(0,1), no overflow risk).

Device layout per (b,h), per dilation branch: the host packs ONE blob
[128, W] per branch holding, in SBUF layout:
  - Q^T [64, Ld] duplicated onto both partition halves (matmul rhs for
    both PE row groups),
  - K^T k-tiles parity-split: even k-tiles on partitions 0-63, odd on
    64-127 (so consecutive K=64 QK matmuls land on different PE row
    groups and run concurrently),
  - V k-tile slabs [128, 65] with a ones column (PV matmul with M=65
    gives the exp row-sum on psum row 64 for free).
One DMA per branch.  S^T = matmul(lhsT=K^T[64,128], rhs=Q^T[64,512]) to
PSUM; exp on ScalarE (PSUM->SBUF, scale=0.125 folds 1/sqrt(hd)); PV
accumulates over k-tiles into a [65, 512] psum window; DVE scatter-adds
windows into per-head accumulators [65, 4096]; 1/w via custom-DVE fast
reciprocal + K=1 ones-matmul partition broadcast; o_proj as 4
accumulating K=64 matmuls per M-tile against Wo^T slices.

Matmul operands are bf16 (fp32 matmuls run as two PE passes on trn2);
psum accumulation and the softmax combine stay fp32.  The PE stream is
software-pipelined (QK/exp of group i+1 issued before PV of group i) so
the in-order PE queue never head-of-line blocks on the ScalarE exp.
"""

import ml_dtypes
import numpy as np

BF16_NP = ml_dtypes.bfloat16

B, H, L, HD = 4, 8, 4096, 64
HIDDEN = H * HD
DILS = (1, 2, 4, 8)
BLOCK = 1024
PB = 4  # (b,h) pairs per core
NCORES = 8
LDS = [L // d for d in DILS]  # 4096, 2048, 1024, 512
NKTS = [ld // 128 for ld in LDS]  # 32, 16, 8, 4
# blob widths per branch: Q dup (Ld) + K parity-split (Ld/2) + V slabs (nkt*65)
WS = [ld + ld // 2 + nkt * 65 for ld, nkt in zip(LDS, NKTS)]
BOFFS = [sum(WS[:i]) for i in range(len(WS))]
WSUM = sum(WS)
QCH = 512  # q-chunk (strided-domain positions) per psum window

_PROGRAM = None


def build_program():
    """Build the (SPMD, identical on all cores) Bass program."""
    from contextlib import ExitStack

    import concourse.tile as tile
    from concourse import bacc, mybir

    F32 = mybir.dt.float32
    BF16 = mybir.dt.bfloat16
    nc = bacc.Bacc("TRN2", target_bir_lowering=False, debug=False)

    blob_d = nc.dram_tensor("blob", [PB, 128, WSUM], BF16, kind="ExternalInput")
    wot_d = nc.dram_tensor("wot", [PB, 64, HIDDEN], BF16, kind="ExternalInput")
    out_d = nc.dram_tensor("out", [L, HIDDEN], F32, kind="ExternalOutput")

    with tile.TileContext(nc) as tc, ExitStack() as ctx:
        consts = ctx.enter_context(tc.tile_pool(name="consts", bufs=1))
        br_pool = ctx.enter_context(tc.tile_pool(name="br", bufs=1))
        e_pool = ctx.enter_context(tc.tile_pool(name="ep", bufs=5))
        acc_pool = ctx.enter_context(tc.tile_pool(name="accp", bufs=1))
        io_pool = ctx.enter_context(tc.tile_pool(name="iop", bufs=2))
        st_psum = ctx.enter_context(tc.tile_pool(name="stp", bufs=3, space="PSUM"))
        pv_psum = ctx.enter_context(tc.tile_pool(name="pvp", bufs=2, space="PSUM"))

        zero_bias = consts.tile([128, 1], F32, tag="zb")
        nc.vector.memset(zero_bias, 0.0)
        ones_row = consts.tile([1, 64], BF16, tag="ones_row")
        nc.vector.memset(ones_row, 1.0)

        wot_sb = consts.tile([64, PB, HIDDEN], BF16, tag="wot")
        nc.sync.dma_start(out=wot_sb, in_=wot_d.rearrange("j r c -> r j c"))

        acc_tiles = [
            acc_pool.tile([65, L], F32, tag=f"acc{j}", bufs=1, name=f"acc{j}")
            for j in range(PB)
        ]
        oacc_tiles = [
            acc_pool.tile([64, L], BF16, tag=f"oacc{j}", bufs=1, name=f"oacc{j}")
            for j in range(PB)
        ]

        for j in range(PB):
            acc = acc_tiles[j]

            # Build the flat job list: one job per (branch, window, k-group).
            jobs = []
            bt_tiles = {}
            for di, d in enumerate(DILS):
                Ld = LDS[di]
                bs = min(BLOCK, Ld)
                nblk = Ld // bs
                nkt_blk = bs // 128
                for blk in range(nblk):
                    for qc in range(bs // QCH):
                        q0 = blk * bs + qc * QCH
                        kts = list(range(nkt_blk))
                        groups = [kts[x : x + 2] for x in range(0, nkt_blk, 2)]
                        for gi, g in enumerate(groups):
                            jobs.append(
                                dict(
                                    di=di,
                                    d=d,
                                    blk=blk,
                                    nkt_blk=nkt_blk,
                                    q0=q0,
                                    g=g,
                                    first=(gi == 0),
                                    last=(gi == len(groups) - 1),
                                    done0=sum(len(x) for x in groups[:gi]),
                                )
                            )

            def get_bt(di):
                if di not in bt_tiles:
                    bufs = 1 if di <= 1 else 2
                    bt = br_pool.tile(
                        [128, WS[di]], BF16, tag=f"b{di}", bufs=bufs, name=f"bt{di}"
                    )
                    nc.sync.dma_start(
                        out=bt, in_=blob_d[j, :, BOFFS[di] : BOFFS[di] + WS[di]]
                    )
                    bt_tiles[di] = bt
                return bt_tiles[di]

            # prefetch the first branches
            get_bt(0)
            get_bt(1)

            def emit_qk_exp(job):
                """QK matmuls for the group -> exp to a bf16 E tile."""
                di, q0, g = job["di"], job["q0"], job["g"]
                Ld = LDS[di]
                kbase = Ld
                bt = get_bt(di)
                gs = len(g)
                st = st_psum.tile([128, 2, QCH], F32, tag="st", name="st")
                for i, kt in enumerate(g):
                    tg = job["blk"] * job["nkt_blk"] + kt
                    half = tg % 2
                    k0 = kbase + (tg // 2) * 128
                    nc.tensor.matmul(
                        st[:, i, :],
                        bt[half * 64 : (half + 1) * 64, k0 : k0 + 128],
                        bt[half * 64 : (half + 1) * 64, q0 : q0 + QCH],
                        start=True,
                        stop=True,
                    )
                et = e_pool.tile([128, 2, QCH], BF16, tag="et", name="et")
                nc.scalar.activation(
                    et[:, 0:gs, :],
                    st[:, 0:gs, :],
                    mybir.ActivationFunctionType.Exp,
                    bias=zero_bias,
                    scale=0.125,
                )
                job["et"] = et

            def emit_pv(job):
                """PV accumulation for the group; combine if window done."""
                di, d = job["di"], job["d"]
                Ld = LDS[di]
                vbase = Ld + Ld // 2
                bt = get_bt(di)
                et = job["et"]
                pv = job["pv"]
                done = job["done0"]
                for i, kt in enumerate(job["g"]):
                    tg = job["blk"] * job["nkt_blk"] + kt
                    nc.tensor.matmul(
                        pv[0:65, :],
                        bt[:, vbase + tg * 65 : vbase + tg * 65 + 65],
                        et[:, i, :],
                        start=(done == 0),
                        stop=(done == job["nkt_blk"] - 1),
                        skip_group_check=True,
                    )
                    done += 1
                if job["last"]:
                    p0 = job["q0"] * d
                    if d == 1:
                        nc.vector.tensor_copy(
                            out=acc[:, p0 : p0 + QCH], in_=pv[0:65, :]
                        )
                    else:
                        dst = acc[:, p0 : p0 + QCH * d : d]
                        nc.vector.tensor_add(out=dst, in0=dst, in1=pv[0:65, :])

            # software pipeline, depth 2: QK/exp of job i, then PV of job
            # i-2, so the in-order PE queue never blocks on the ACT exp.
            from collections import deque

            pending = deque()
            cur_pv = None
            for idx, job in enumerate(jobs):
                if job["first"]:
                    cur_pv = pv_psum.tile([128, QCH], F32, tag="pv", name="pv")
                job["pv"] = cur_pv
                # prefetch next branch blob one branch ahead
                if idx > 0 and job["di"] != jobs[idx - 1]["di"] and job["di"] < 3:
                    get_bt(job["di"] + 1)
                emit_qk_exp(job)
                pending.append(job)
                if len(pending) > 3:
                    emit_pv(pending.popleft())
            while pending:
                emit_pv(pending.popleft())
            bt_tiles.clear()

            # normalize: oacc = acc[0:64, :] * (1 / acc[64, :]) in bf16.
            # Custom-DVE ops silently no-op at base partition 64, so move the
            # w row to partition 0 first (SBUF->SBUF DMA crosses partitions).
            wrow = io_pool.tile([1, L], F32, tag="wrow", bufs=1)
            nc.sync.dma_start(out=wrow, in_=acc[64:65, :])
            nc.vector.reciprocal_approx_fast(out=wrow, in_=wrow)
            wrowb = io_pool.tile([1, L], BF16, tag="wrowb", bufs=1)
            nc.vector.tensor_copy(out=wrowb, in_=wrow)
            for w in range(L // QCH):
                ws = slice(w * QCH, (w + 1) * QCH)
                # broadcast 1/w across 64 partitions via a K=1 ones-matmul
                bc = pv_psum.tile([64, QCH], F32, tag="pv", name="bc")
                nc.tensor.matmul(
                    bc, ones_row[0:1, :], wrowb[0:1, ws], start=True, stop=True
                )
                nc.vector.tensor_mul(
                    out=oacc_tiles[j][:, ws], in0=acc[0:64, ws], in1=bc
                )

        # partial o_proj: out[p, :] = sum_j oacc_j[:, p]^T @ wot_j
        for mt in range(L // 128):
            po = pv_psum.tile([128, HIDDEN], F32, tag="pv", name="po")
            for j in range(PB):
                nc.tensor.matmul(
                    po,
                    oacc_tiles[j][:, mt * 128 : (mt + 1) * 128],
                    wot_sb[:, j, :],
                    start=(j == 0),
                    stop=(j == PB - 1),
                    skip_group_check=True,
                )
            ot = io_pool.tile([128, HIDDEN], F32, tag="ot")
            nc.vector.tensor_copy(out=ot, in_=po)
            nc.sync.dma_start(out=out_d[mt * 128 : (mt + 1) * 128, :], in_=ot)

    nc.compile()
    return nc


def get_program():
    global _PROGRAM
    if _PROGRAM is None:
        _PROGRAM = build_program()
    return _PROGRAM


def _branch_blob(qT, kT, vv, di):
    """Pack one dilation branch into the [128, W] SBUF-layout blob.

    qT, kT: [64, Ld] transposed Q/K for this branch; vv: [Ld, 65] V plus
    ones column."""
    Ld, nkt = LDS[di], NKTS[di]
    q_part = np.concatenate([qT, qT], axis=0)  # [128, Ld]
    k3 = kT.reshape(64, nkt, 128)
    k_part = np.concatenate(
        [
            k3[:, 0::2, :].reshape(64, -1),
            k3[:, 1::2, :].reshape(64, -1),
        ],
        axis=0,
    )  # [128, Ld/2]
    v_part = vv.reshape(nkt, 128, 65).transpose(1, 0, 2).reshape(128, nkt * 65)
    return np.concatenate([q_part, k_part, v_part], axis=1)


def make_in_maps(query_states, key_states, value_states, Wo):
    q = np.asarray(query_states, dtype=np.float32)
    k = np.asarray(key_states, dtype=np.float32)
    v = np.asarray(value_states, dtype=np.float32)
    Wo = np.asarray(Wo, dtype=np.float32)

    in_maps = []
    for c in range(NCORES):
        b, hs = c // 2, (c % 2) * PB
        blob = np.empty((PB, 128, WSUM), BF16_NP)
        wot = np.empty((PB, 64, HIDDEN), BF16_NP)
        for j in range(PB):
            h = hs + j
            for di, d in enumerate(DILS):
                Ld = LDS[di]
                vv = np.empty((Ld, 65), np.float32)
                vv[:, 0:64] = v[b, h, ::d, :]
                vv[:, 64] = 1.0
                blob[j, :, BOFFS[di] : BOFFS[di] + WS[di]] = _branch_blob(
                    np.ascontiguousarray(q[b, h, ::d, :].T),
                    np.ascontiguousarray(k[b, h, ::d, :].T),
                    vv,
                    di,
                )
            wot[j] = Wo[:, h * 64 : (h + 1) * 64].T
        in_maps.append({"blob": blob, "wot": wot})
    return in_maps


def combine_outputs(results, bo):
    bo = np.asarray(bo, dtype=np.float32)
    out = np.empty((B, L, HIDDEN), np.float32)
    for b in range(B):
        out[b] = results[2 * b]["out"] + results[2 * b + 1]["out"] + bo
    return out


def kernel(
    query_states,
    key_states,
    value_states,
    Wo,
    bo,
    _trace=False,
    _tmpdir=None,
    _results=[None],
):
    from concourse.bass_utils import run_bass_kernel_spmd

    nc = get_program()
    in_maps = make_in_maps(query_states, key_states, value_states, Wo)
    res = run_bass_kernel_spmd(
        nc, in_maps, list(range(NCORES)), trace=_trace, tmpdir=_tmpdir
    )
    _results[0] = res
    return combine_outputs(res.results, bo)
